# revision 1
# baseline (speedup 1.0000x reference)
"""TRN2 Bass kernel for nn_BlockLinear: per token t (32768 of them),
x_t [32,128] -> P(P(x_t@w1)@w2) where P(Y) = reshape(Y.T, (32,128)).

Strategy (data-parallel over 8 NeuronCores, 4096 tokens/core):
  - Host: round x/w to fp32r (12-bit significand; PE runs fp32r matmuls at
    bf16 speed), permute weight columns so the inter-stage permutation
    becomes a 32x32 blockwise transpose (DVE native op).
  - Layout: one whole token per SBUF partition at both DMA ends -> 16 KiB
    contiguous HBM runs (full DMA rate). Token structure is recovered
    on-chip with PE transposes.
  - Per 128-token chunk (2 MiB):
      load XB[tok, (b,m)] -> 32x PE-T -> Xt[m, (T,b)] -> mm1(w1p) ->
      DVE 32x32-T -> fp32r cast -> mm2(w2p) -> DVE 32x32-T (restrided) ->
      32x PE-T -> OB[tok, (i,j)] -> store.
"""
import numpy as np
from contextlib import ExitStack

import concourse.bass as bass
from concourse import bacc
import concourse.tile as tile
from concourse import mybir
from concourse.bass_utils import run_bass_kernel_spmd

F32 = mybir.dt.float32
F32R = mybir.dt.float32r

N_CORES = 8
TOK_PER_CORE = 4096
CHUNK_TOK = 128          # one token per partition
N = 4096                 # elems per token


def _round_f32r(a):
    u = np.ascontiguousarray(a).view(np.uint32)
    r = ((u.astype(np.uint64) + 0x800) & 0xFFFFF000).astype(np.uint32)
    return r.view(np.float32)


def _perm():
    p = np.zeros(128, np.int64)
    for h in range(4):
        for i in range(32):
            p[32 * h + i] = 4 * i + h
    return p


def build_nc(ntok):
    nchunks = ntok // CHUNK_TOK
    nc = bacc.Bacc("TRN2", target_bir_lowering=False, debug=False)
    X = nc.dram_tensor("x", [ntok, N], F32R, kind="ExternalInput").ap()
    W1 = nc.dram_tensor("w1p", [128, 128], F32R, kind="ExternalInput").ap()
    W2 = nc.dram_tensor("w2p", [128, 128], F32R, kind="ExternalInput").ap()
    IR = nc.dram_tensor("ident_r", [128, 128], F32R, kind="ExternalInput").ap()
    IF = nc.dram_tensor("ident_f", [128, 128], F32, kind="ExternalInput").ap()
    OUT = nc.dram_tensor("out", [ntok, N], F32, kind="ExternalOutput").ap()

    with tile.TileContext(nc) as tc, ExitStack() as ctx:
        wpool = ctx.enter_context(tc.tile_pool(name="w", bufs=1))
        xbp = ctx.enter_context(tc.tile_pool(name="xbp", bufs=3))
        zrp = ctx.enter_context(tc.tile_pool(name="zrp", bufs=3))
        xtp = ctx.enter_context(tc.tile_pool(name="xtp", bufs=2))
        ztp = ctx.enter_context(tc.tile_pool(name="ztp", bufs=2))
        gp_ = ctx.enter_context(tc.tile_pool(name="gp", bufs=2))
        obp = ctx.enter_context(tc.tile_pool(name="obp", bufs=2))
        # PSUM: tag "a" shared {T_in, mm1}, tag "b" shared {mm2, T_out}.
        # Both pairings follow chunk program order (no cross-chunk stalls).
        psp = ctx.enter_context(tc.tile_pool(name="psp", bufs=2, space="PSUM"))

        w1_sb = wpool.tile([128, 128], F32R)
        w2_sb = wpool.tile([128, 128], F32R)
        ir_sb = wpool.tile([128, 128], F32R)
        if_sb = wpool.tile([128, 128], F32)
        nc.sync.dma_start(w1_sb[:], W1[:])
        nc.sync.dma_start(w2_sb[:], W2[:])
        nc.sync.dma_start(ir_sb[:], IR[:])
        nc.sync.dma_start(if_sb[:], IF[:])

        for c in range(nchunks):
            # 1. load: XB[p, (b,m)] = x[c*128 + p, b, m]; 16 KiB/partition
            xb = xbp.tile([128, N], F32R, tag="xb")
            nc.sync.dma_start(xb[:], X[c * CHUNK_TOK:(c + 1) * CHUNK_TOK, :])

            # 2./3. T_in (groups of 8) + scatter-evac at FD=1024: Xt[m, 32T+b]
            xt = xtp.tile([128, N], F32R, tag="xt")
            for b8 in range(4):
                tin = psp.tile([128, 1024], F32R, tag="a")
                for bb in range(8):
                    b = 8 * b8 + bb
                    nc.tensor.transpose(
                        tin[:, bass.ts(bb, 128)], xb[:, bass.ts(b, 128)], ir_sb[:]
                    )
                # out positions 32T + 8*b8 + bb, iterated (T, bb): runs-of-8
                # writes + strided PSUM reads (measured same cost as contiguous)
                dst = xt[:].rearrange("p (t g b) -> p g t b", g=4, b=8)[:, b8, :, :]
                src = tin[:].rearrange("p (b t) -> p t b", b=8)
                nc.scalar.copy(dst, src)

            # 4./5. mm1 + VT1 at FD=1024 -> zraw (fp32), then 6. cast to f32r
            zt = ztp.tile([128, N], F32R, tag="zt")
            for q2 in range(4):
                y1 = psp.tile([128, 1024], F32, tag="a")
                for qq in range(2):
                    q = 2 * q2 + qq
                    nc.tensor.matmul(y1[:, bass.ts(qq, 512)], w1_sb[:],
                                     xt[:, bass.ts(q, 512)], start=True, stop=True)
                zraw = zrp.tile([128, 1024], F32, tag="zraw")
                nc.vector.transpose(zraw[:], y1[:])
                # cast split: gpsimd x2, scalar x1, vector x1 per chunk
                dst = zt[:, bass.ts(q2, 1024)]
                if q2 < 2:
                    nc.gpsimd.tensor_copy(dst, zraw[:])
                elif q2 == 2:
                    nc.scalar.copy(dst, zraw[:])
                else:
                    nc.vector.tensor_copy(dst, zraw[:])

            # 7./8. mm2 + VT2 at FD=1024 (contiguous out): G[j, 32T + i]
            g = gp_.tile([128, N], F32, tag="g")
            for q2 in range(4):
                y2 = psp.tile([128, 1024], F32, tag="b")
                for qq in range(2):
                    q = 2 * q2 + qq
                    nc.tensor.matmul(y2[:, bass.ts(qq, 512)], w2_sb[:],
                                     zt[:, bass.ts(q, 512)], start=True, stop=True)
                nc.vector.transpose(g[:, bass.ts(q2, 1024)], y2[:])

            # 9./10. T_out (strided lhsT: U_i = G[:, i::32]) + evac2 FD=1024
            ob = obp.tile([128, N], F32, tag="ob")
            gv = g[:].rearrange("p (t i) -> p i t", i=32)  # [p, i(32), T(128)]
            for i8 in range(4):
                tout = psp.tile([128, 1024], F32, tag="b")
                for ii in range(8):
                    i = 8 * i8 + ii
                    nc.tensor.transpose(
                        tout[:, bass.ts(ii, 128)], gv[:, i, :], if_sb[:]
                    )
                nc.scalar.copy(ob[:, bass.ts(i8, 1024)], tout[:])

            # 11. store
            nc.sync.dma_start(OUT[c * CHUNK_TOK:(c + 1) * CHUNK_TOK, :], ob[:])

    if not nc.is_finalized():
        nc.finalize()
    return nc


_NC_CACHE = {}


def _get_nc(ntok):
    if ntok not in _NC_CACHE:
        _NC_CACHE[ntok] = build_nc(ntok)
    return _NC_CACHE[ntok]


def kernel(x, w1, w2):
    """x [8, 4096, 4096] f32; w1, w2 [128, 128] f32 -> [8, 4096, 4096] f32."""
    lead = x.shape[:-1]
    xf = np.ascontiguousarray(x, dtype=np.float32).reshape(-1, N)
    ntok_total = xf.shape[0]
    assert ntok_total % N_CORES == 0
    ntok = ntok_total // N_CORES

    perm = _perm()
    w1p = _round_f32r(np.ascontiguousarray(w1, np.float32)[:, perm])
    w2p = _round_f32r(np.ascontiguousarray(w2, np.float32)[:, perm])
    ident = np.eye(128, dtype=np.float32)
    xr = _round_f32r(xf)

    nc = _get_nc(ntok)
    in_maps = []
    for i in range(N_CORES):
        in_maps.append({
            "x": xr[i * ntok:(i + 1) * ntok],
            "w1p": w1p, "w2p": w2p,
            "ident_r": ident, "ident_f": ident,
        })
    res = run_bass_kernel_spmd(nc, in_maps, list(range(N_CORES)))
    out = np.empty((ntok_total, N), np.float32)
    for i in range(N_CORES):
        out[i * ntok:(i + 1) * ntok] = res.results[i]["out"]
    return out.reshape(*lead, N)



# revision 11
# speedup vs baseline: 1.1440x; 1.1440x over previous
"""TRN2 Bass kernel for nn_BlockLinear: per token t, x_t [32,128] ->
P(P(x_t@w1)@w2) where P(Y) = reshape(Y.T, (32,128)).

v2 strategy (data-parallel over 8 NeuronCores, 4096 tokens/core):
  - All wire traffic in bf16 (halves HBM bytes; rel-err budget 2e-2 allows it).
  - Host pre-transposes x to XT[k, (t,b)] so the contraction dim k is on
    partitions at DMA time -> NO on-device input transpose.  Host also
    absorbs the final P permutation on download -> NO on-device output
    transpose.  Only the mid-stage P runs on-device.
  - Weight columns are permuted (w*p = w*[:, perm], perm[32a+i] = 4i+a) so
    the mid-stage P becomes a 32x32 diagonal-block transpose (DVE native).
  - Per 256-token chunk: DMA in -> 16x matmul(w1p) -> 8x DVE block-transpose
    (PSUM f32 -> SBUF f32; ISA requires same src/dst dtype) -> 8x GpSimd
    cast f32->bf16 (SBUF->SBUF; GpSimd cannot touch PSUM) -> 16x
    matmul(w2p) -> 8x Act cast-evac -> DMA out.  Chunk-level software
    pipeline (stage1 of chunk c+1 before stage2 of chunk c) keeps PE fed.
"""
import numpy as np
import ml_dtypes
from contextlib import ExitStack

import concourse.bass as bass
from concourse import bacc
import concourse.tile as tile
from concourse import mybir
from concourse.bass_utils import run_bass_kernel_spmd

F32 = mybir.dt.float32
BF16 = mybir.dt.bfloat16

N_CORES = 8
TOK_PER_CORE = 4096
CHUNK_TOK = 256          # tokens per chunk; free dim = 32*256 = 8192
N = 4096                 # elems per token


def _perm():
    p = np.zeros(128, np.int64)
    for a in range(4):
        for i in range(32):
            p[32 * a + i] = 4 * i + a
    return p


def _f32_to_bf16_u16(a):
    """Round-to-nearest-even f32 -> bf16 bit pattern (uint16)."""
    u = np.ascontiguousarray(a, np.float32).view(np.uint32)
    r = ((u.astype(np.uint64) + 0x7FFF + ((u >> 16) & 1)) >> 16).astype(np.uint16)
    return r


def _u16_to_f32(u):
    return (u.astype(np.uint32) << 16).view(np.float32)


def build_nc(ntok):
    nchunks = ntok // CHUNK_TOK
    FD = CHUNK_TOK * 32  # free-dim elems per chunk
    nc = bacc.Bacc("TRN2", target_bir_lowering=False, debug=False)
    X = nc.dram_tensor("xt", [128, ntok * 32], BF16, kind="ExternalInput").ap()
    W1 = nc.dram_tensor("w1p", [128, 128], BF16, kind="ExternalInput").ap()
    W2 = nc.dram_tensor("w2p", [128, 128], BF16, kind="ExternalInput").ap()
    OUT = nc.dram_tensor("out", [128, ntok * 32], BF16, kind="ExternalOutput").ap()

    with tile.TileContext(nc) as tc, ExitStack() as ctx:
        wpool = ctx.enter_context(tc.tile_pool(name="w", bufs=1))
        xtp = ctx.enter_context(tc.tile_pool(name="xtp", bufs=2))
        mrp = ctx.enter_context(tc.tile_pool(name="mrp", bufs=2))
        m2p = ctx.enter_context(tc.tile_pool(name="m2p", bufs=2))
        obp = ctx.enter_context(tc.tile_pool(name="obp", bufs=2))
        psp = ctx.enter_context(tc.tile_pool(name="psp", bufs=2, space="PSUM"))

        w1_sb = wpool.tile([128, 128], BF16)
        w2_sb = wpool.tile([128, 128], BF16)
        nc.sync.dma_start(w1_sb[:], W1[:])
        nc.sync.dma_start(w2_sb[:], W2[:])

        m_tiles = {}

        def stage1(c):
            xt = xtp.tile([128, FD], BF16, tag="xt")
            nc.sync.dma_start(xt[:], X[:, c * FD:(c + 1) * FD])
            mraw = mrp.tile([128, FD], F32, tag="mraw")
            m2 = m2p.tile([128, FD], BF16, tag="m2")
            for q in range(FD // 1024):
                ps = psp.tile([128, 1024], F32, tag="a")
                nc.tensor.matmul(ps[:, 0:512], w1_sb[:],
                                 xt[:, bass.ts(2 * q, 512)], start=True, stop=True)
                nc.tensor.matmul(ps[:, 512:1024], w1_sb[:],
                                 xt[:, bass.ts(2 * q + 1, 512)], start=True, stop=True)
                nc.vector.transpose(mraw[:, bass.ts(q, 1024)], ps[:])
                nc.gpsimd.tensor_copy(m2[:, bass.ts(q, 1024)],
                                      mraw[:, bass.ts(q, 1024)])
            m_tiles[c] = m2

        def stage2(c):
            m2 = m_tiles.pop(c)
            ob = obp.tile([128, FD], BF16, tag="ob")
            for q in range(FD // 1024):
                ps = psp.tile([128, 1024], F32, tag="b")
                nc.tensor.matmul(ps[:, 0:512], w2_sb[:],
                                 m2[:, bass.ts(2 * q, 512)], start=True, stop=True)
                nc.tensor.matmul(ps[:, 512:1024], w2_sb[:],
                                 m2[:, bass.ts(2 * q + 1, 512)], start=True, stop=True)
                nc.scalar.copy(ob[:, bass.ts(q, 1024)], ps[:])
            nc.sync.dma_start(OUT[:, c * FD:(c + 1) * FD], ob[:])

        for c in range(nchunks + 1):
            if c < nchunks:
                stage1(c)
            if c >= 1:
                stage2(c - 1)

    if not nc.is_finalized():
        nc.finalize()
    return nc


_NC_CACHE = {}


def _get_nc(ntok):
    if ntok not in _NC_CACHE:
        _NC_CACHE[ntok] = build_nc(ntok)
    return _NC_CACHE[ntok]


def prepare_in_maps(x, w1, w2):
    """Host-side shard + layout transform. Returns (in_maps, ntok)."""
    xf = np.ascontiguousarray(x, dtype=np.float32).reshape(-1, N)
    ntok_total = xf.shape[0]
    assert ntok_total % N_CORES == 0
    ntok = ntok_total // N_CORES

    perm = _perm()
    w1p = _f32_to_bf16_u16(np.ascontiguousarray(w1, np.float32)[:, perm])
    w2p = _f32_to_bf16_u16(np.ascontiguousarray(w2, np.float32)[:, perm])
    w1p = w1p.view(ml_dtypes.bfloat16)
    w2p = w2p.view(ml_dtypes.bfloat16)

    xu = _f32_to_bf16_u16(xf)  # [T, 4096] u16
    in_maps = []
    for i in range(N_CORES):
        xc = xu[i * ntok:(i + 1) * ntok].reshape(ntok, 32, 128)
        # XT[k, 32 t + b] = x[t, 128 b + k]
        xt = np.ascontiguousarray(xc.transpose(2, 0, 1)).reshape(128, ntok * 32)
        in_maps.append({
            "xt": xt.view(ml_dtypes.bfloat16),
            "w1p": w1p, "w2p": w2p,
        })
    return in_maps, ntok


def postprocess(results, ntok, lead):
    """Gather per-core OT [128, ntok*32] bf16 -> full f32 output."""
    ntok_total = ntok * N_CORES
    out = np.empty((ntok_total, N), np.float32)
    for i in range(N_CORES):
        ot = np.asarray(results[i]["out"]).view(np.uint16)
        # out[t, 128 i2 + 32 a2 + b2] = OT[32 a2 + i2, 32 t + b2]
        ot = ot.reshape(4, 32, ntok, 32)              # [a2, i2, t, b2]
        oc = ot.transpose(2, 1, 0, 3).reshape(ntok, N)
        out[i * ntok:(i + 1) * ntok] = _u16_to_f32(np.ascontiguousarray(oc))
    return out.reshape(*lead, N)


def kernel(x, w1, w2):
    """x [8, 4096, 4096] f32; w1, w2 [128, 128] f32 -> [8, 4096, 4096] f32."""
    lead = x.shape[:-1]
    in_maps, ntok = prepare_in_maps(x, w1, w2)
    nc = _get_nc(ntok)
    res = run_bass_kernel_spmd(nc, in_maps, list(range(N_CORES)))
    return postprocess(res.results, ntok, lead)


# revision 12
# speedup vs baseline: 2.0771x; 1.8157x over previous
"""TRN2 Bass kernel for nn_BlockLinear: per token t, x_t [32,128] ->
P(P(x_t@w1)@w2) where P(Y) = reshape(Y.T, (32,128)).

v3 strategy (data-parallel over 8 NeuronCores, 4096 tokens/core):
  - All wire traffic in bf16 (halves HBM bytes; rel-err budget 2e-2 allows it).
  - Host pre-transposes x to XT[k, (tau,b,h)] with t = 2*tau + h so the
    contraction dim k is on partitions at DMA time -> NO on-device input
    transpose.  Host also absorbs the final P permutation on download ->
    NO on-device output transpose.  Only the mid-stage P runs on-device.
  - Weight columns are permuted (w*p = w*[:, perm], perm[32a+i] = 4i+a) so
    the mid-stage P becomes a 32x32 diagonal-block transpose (DVE native).
    The adjacent-token pair (h=0/1) of a given (b) occupies one aligned
    bf16 pair and travels to the same destination partition, so the DVE
    transpose runs on an int32 view -> half the DVE cycles.
  - Per 256-token chunk: DMA in -> 16x matmul(w1p) -> 8x cast-evac PSUM
    f32 -> SBUF bf16 (split Act/DVE; GpSimd cannot touch PSUM and its
    casts are ~3.5us/tile) -> 2x DVE int32 block-transpose -> 16x
    matmul(w2p) -> 8x cast-evac (Act/DVE) -> DMA out.  Chunk-level
    software pipeline (stage1 of chunk c+1 before stage2 of chunk c)
    keeps PE warm while DVE/Act drain chunk c.
"""
import numpy as np
import ml_dtypes
from contextlib import ExitStack

import concourse.bass as bass
from concourse import bacc
import concourse.tile as tile
from concourse import mybir
from concourse.bass_utils import run_bass_kernel_spmd

F32 = mybir.dt.float32
BF16 = mybir.dt.bfloat16
I32 = mybir.dt.int32

N_CORES = 8
TOK_PER_CORE = 4096
CHUNK_TOK = 256          # tokens per chunk; free dim = 32*256 = 8192
N = 4096                 # elems per token

# engine split for the 8+8 PSUM->SBUF cast-evac tiles per chunk
# (Act ~997ns/tile, DVE ~1192ns/tile; DVE also runs ~4.4us of transposes)
EVAC1 = ("act", "dve", "act", "act", "dve", "act", "act", "dve")  # 5A/3D
EVAC2 = ("act", "dve", "act", "act", "act", "dve", "act", "act")  # 6A/2D


def _perm():
    p = np.zeros(128, np.int64)
    for a in range(4):
        for i in range(32):
            p[32 * a + i] = 4 * i + a
    return p


def _f32_to_bf16_u16(a):
    """Round-to-nearest-even f32 -> bf16 bit pattern (uint16)."""
    u = np.ascontiguousarray(a, np.float32).view(np.uint32)
    r = ((u.astype(np.uint64) + 0x7FFF + ((u >> 16) & 1)) >> 16).astype(np.uint16)
    return r


def _u16_to_f32(u):
    return (u.astype(np.uint32) << 16).view(np.float32)


def build_nc(ntok):
    nchunks = ntok // CHUNK_TOK
    FD = CHUNK_TOK * 32  # free-dim elems per chunk
    nc = bacc.Bacc("TRN2", target_bir_lowering=False, debug=False)
    X = nc.dram_tensor("xt", [128, ntok * 32], BF16, kind="ExternalInput").ap()
    W1 = nc.dram_tensor("w1p", [128, 128], BF16, kind="ExternalInput").ap()
    W2 = nc.dram_tensor("w2p", [128, 128], BF16, kind="ExternalInput").ap()
    OUT = nc.dram_tensor("out", [128, ntok * 32], BF16, kind="ExternalOutput").ap()

    def evac(kind, dst, src):
        if kind == "act":
            nc.scalar.copy(dst, src)
        else:
            nc.vector.tensor_copy(dst, src)

    with tile.TileContext(nc) as tc, ExitStack() as ctx:
        wpool = ctx.enter_context(tc.tile_pool(name="w", bufs=1))
        xtp = ctx.enter_context(tc.tile_pool(name="xtp", bufs=2))
        y1p = ctx.enter_context(tc.tile_pool(name="y1p", bufs=2))
        m2p = ctx.enter_context(tc.tile_pool(name="m2p", bufs=2))
        obp = ctx.enter_context(tc.tile_pool(name="obp", bufs=2))
        psp = ctx.enter_context(tc.tile_pool(name="psp", bufs=2, space="PSUM"))

        w1_sb = wpool.tile([128, 128], BF16)
        w2_sb = wpool.tile([128, 128], BF16)
        nc.sync.dma_start(w1_sb[:], W1[:])
        nc.sync.dma_start(w2_sb[:], W2[:])

        m_tiles = {}

        def stage1(c):
            xt = xtp.tile([128, FD], BF16, tag="xt")
            nc.sync.dma_start(xt[:], X[:, c * FD:(c + 1) * FD])
            y1e = y1p.tile([128, FD], BF16, tag="y1e")
            for q in range(FD // 1024):
                ps = psp.tile([128, 1024], F32, tag="a")
                nc.tensor.matmul(ps[:, 0:512], w1_sb[:],
                                 xt[:, bass.ts(2 * q, 512)], start=True, stop=True)
                nc.tensor.matmul(ps[:, 512:1024], w1_sb[:],
                                 xt[:, bass.ts(2 * q + 1, 512)], start=True, stop=True)
                evac(EVAC1[q % 8], y1e[:, bass.ts(q, 1024)], ps[:])
            m2 = m2p.tile([128, FD], BF16, tag="m2")
            for hh in range(2):
                dst = m2[:, bass.ts(hh, FD // 2)].bitcast(I32)
                src = y1e[:, bass.ts(hh, FD // 2)].bitcast(I32)
                nc.vector.transpose(dst, src)
            m_tiles[c] = m2

        def stage2(c):
            m2 = m_tiles.pop(c)
            ob = obp.tile([128, FD], BF16, tag="ob")
            for q in range(FD // 1024):
                ps = psp.tile([128, 1024], F32, tag="b")
                nc.tensor.matmul(ps[:, 0:512], w2_sb[:],
                                 m2[:, bass.ts(2 * q, 512)], start=True, stop=True)
                nc.tensor.matmul(ps[:, 512:1024], w2_sb[:],
                                 m2[:, bass.ts(2 * q + 1, 512)], start=True, stop=True)
                evac(EVAC2[q % 8], ob[:, bass.ts(q, 1024)], ps[:])
            nc.sync.dma_start(OUT[:, c * FD:(c + 1) * FD], ob[:])

        for c in range(nchunks + 1):
            if c < nchunks:
                stage1(c)
            if c >= 1:
                stage2(c - 1)

    if not nc.is_finalized():
        nc.finalize()
    return nc


_NC_CACHE = {}


def _get_nc(ntok):
    if ntok not in _NC_CACHE:
        _NC_CACHE[ntok] = build_nc(ntok)
    return _NC_CACHE[ntok]


def prepare_in_maps(x, w1, w2):
    """Host-side shard + layout transform. Returns (in_maps, ntok)."""
    xf = np.ascontiguousarray(x, dtype=np.float32).reshape(-1, N)
    ntok_total = xf.shape[0]
    assert ntok_total % N_CORES == 0
    ntok = ntok_total // N_CORES

    perm = _perm()
    w1p = _f32_to_bf16_u16(np.ascontiguousarray(w1, np.float32)[:, perm])
    w2p = _f32_to_bf16_u16(np.ascontiguousarray(w2, np.float32)[:, perm])
    w1p = w1p.view(ml_dtypes.bfloat16)
    w2p = w2p.view(ml_dtypes.bfloat16)

    xu = _f32_to_bf16_u16(xf)  # [T, 4096] u16
    in_maps = []
    for i in range(N_CORES):
        xc = xu[i * ntok:(i + 1) * ntok].reshape(ntok // 2, 2, 32, 128)
        # XT[k, tau*64 + 2b + h] = x[2 tau + h, 128 b + k]
        xt = np.ascontiguousarray(xc.transpose(3, 0, 2, 1)).reshape(128, ntok * 32)
        in_maps.append({
            "xt": xt.view(ml_dtypes.bfloat16),
            "w1p": w1p, "w2p": w2p,
        })
    return in_maps, ntok


def postprocess(results, ntok, lead):
    """Gather per-core OT [128, ntok*32] bf16 -> full f32 output."""
    ntok_total = ntok * N_CORES
    out = np.empty((ntok_total, N), np.float32)
    for i in range(N_CORES):
        ot = np.asarray(results[i]["out"]).view(np.uint16)
        # out[2 tau + h, 128 i2 + 32 a2 + b2] = OT[32 a2 + i2, tau*64 + 2 b2 + h]
        ot = ot.reshape(4, 32, ntok // 2, 32, 2)      # [a2, i2, tau, b2, h]
        oc = ot.transpose(2, 4, 1, 0, 3).reshape(ntok, N)
        out[i * ntok:(i + 1) * ntok] = _u16_to_f32(np.ascontiguousarray(oc))
    return out.reshape(*lead, N)


def kernel(x, w1, w2):
    """x [8, 4096, 4096] f32; w1, w2 [128, 128] f32 -> [8, 4096, 4096] f32."""
    lead = x.shape[:-1]
    in_maps, ntok = prepare_in_maps(x, w1, w2)
    nc = _get_nc(ntok)
    res = run_bass_kernel_spmd(nc, in_maps, list(range(N_CORES)))
    return postprocess(res.results, ntok, lead)
